# revision 24
# baseline (speedup 1.0000x reference)
"""Causal multi-head attention (B=2, S=2048, D=2048, 32 heads x 64) for 8
Trainium2 NeuronCores.

Sharding: data parallel on batch (2 groups of 4 cores) x tensor parallel on
heads (4 groups of 8 heads each). Each core computes q/k/v projections for
its head group, RoPE, causal attention with sigmoid-gated values, and a
partial o-projection; the host sums the 4 partials per batch (the
"all-reduce" of the o-projection) and adds the output bias.

v3 design (vs the f32r baseline at ~608us):
- All matmul operands bf16 (1 cyc/row at any N); PSUM accumulates fp32.
- x loaded ONCE per half; V projection folded into the per-half loop.
- Attention for query blocks 0-1 (which needs only half-0 q/k/v) is
  interleaved into the half-1 projection stream, so the Act engine's
  softmax-exp work hides under projection matmuls instead of
  serializing after them.
- Sigmoid gate computed as tanh(v/2) = 2*sigmoid(v)-1 (same activation
  table as exp -> zero table switches); the 0.5 factor folds into Wo
  and the +0.5 mean term becomes a host-side constant on the output.
  Centering also halves the bf16 quantization error of the gated
  values.
- Scores matmuls contract 64 partitions directly; causal masking is a
  PE identity-matmul accumulation (width-sliced); exp processes two key
  tiles per activation; one strided tanh per 128-row v tile.
"""

import os

import numpy as np
import ml_dtypes

import concourse.bacc as bacc
import concourse.tile as tile
from concourse import mybir
from concourse.bass_utils import run_bass_kernel_spmd

B, S, D = 2, 2048, 2048
H_PER_CORE = 8          # heads per core
DH = 64                 # head dim
CW = 512                # per-core projection width = H_PER_CORE * DH
N_CORES = 8
KT = D // 128           # k-subtiles for the D-contraction

f32 = mybir.dt.float32
bf16 = mybir.dt.bfloat16
Act = mybir.ActivationFunctionType

TRACE = bool(int(os.environ.get("KERNEL_TRACE", "0")))
LAST_EXEC_NS = None
LAST_MEAN_NS = None

_SENT = object()


def _build(WITH_BIAS=True):
    nc = bacc.Bacc("TRN2", target_bir_lowering=False, debug=False)

    x4 = nc.dram_tensor("x4", [4, 128, KT, 512], bf16, kind="ExternalInput")
    wq4 = nc.dram_tensor("wq4", [4, 128, KT, 128], bf16, kind="ExternalInput")
    wk4 = nc.dram_tensor("wk4", [4, 128, KT, 128], bf16, kind="ExternalInput")
    wv4 = nc.dram_tensor("wv4", [128, KT, CW], bf16, kind="ExternalInput")
    wo4 = nc.dram_tensor("wo4", [4, 128, 4, 512], bf16, kind="ExternalInput")
    bq = nc.dram_tensor("bq", [1, CW], bf16, kind="ExternalInput")
    bk = nc.dram_tensor("bk", [1, CW], bf16, kind="ExternalInput")
    bv = nc.dram_tensor("bv", [1, CW], bf16, kind="ExternalInput")
    ropec = nc.dram_tensor("ropec", [4, 128, 512], f32, kind="ExternalInput")
    ropes = nc.dram_tensor("ropes", [4, 64, 512], f32, kind="ExternalInput")
    masks = nc.dram_tensor("masks", [128, 4, 512], bf16, kind="ExternalInput")
    ident = nc.dram_tensor("ident", [128, 128], bf16, kind="ExternalInput")
    vinit = nc.dram_tensor("vinit", [128, 16 * 520], bf16, kind="ExternalInput")
    part = nc.dram_tensor("part", [S, D], f32, kind="ExternalOutput")

    with tile.TileContext(nc) as tc:
        with (
            tc.tile_pool(name="p0", bufs=1) as p0,
            tc.tile_pool(name="pqk", bufs=1) as pqk,
            tc.tile_pool(name="py", bufs=1) as py,
        ):
            # persistent state
            qt_all = pqk.tile([128, 4, S], bf16, name="qt_all")
            kt_all = pqk.tile([128, 4, S], bf16, name="kt_all")
            qt = [qt_all[:, i, :] for i in range(4)]
            kt = [kt_all[:, i, :] for i in range(4)]
            va_all = p0.tile([128, 16 * 520 + 4 * CW], bf16, name="va_all")
            va = [va_all[:, 520 * i:520 * (i + 1)] for i in range(16)]
            ones = va_all[0:1, 8320:8320 + CW]
            bvt = va_all[0:1, 8832:8832 + CW]
            bqrow = va_all[0:1, 9344:9344 + CW]
            bkrow = va_all[0:1, 9856:9856 + CW]
            idt = p0.tile([128, 128], bf16, name="idt")
            maskt = p0.tile([128, 4, 512], bf16, name="maskt")
            wvf = p0.tile([128, KT, CW], bf16, name="wvf")
            ytr = [py.tile([128, S], bf16, name=f"ytr{i}") for i in range(4)]

            def load_xh(half, pa):
                xh = pa.tile([128, 2, KT, 512], bf16, tag="xh", name="xh")
                for qloc in range(2):
                    for kg in range(4):
                        nc.sync.dma_start(
                            xh[:, qloc, 4 * kg:4 * kg + 4, :],
                            x4[2 * half + qloc, :, 4 * kg:4 * kg + 4, :])
                return xh

            def load_rope(half, prc):
                cosw = prc.tile([128, 2, 512], f32, tag="tblc", name="cosw")
                rsnw = prc.tile([64, 2, 512], f32, tag="tbls", name="rsnw")
                for qloc in range(2):
                    nc.sync.dma_start(cosw[:, qloc, :], ropec[2 * half + qloc])
                    nc.sync.dma_start(rsnw[:, qloc, :], ropes[2 * half + qloc])
                return cosw, rsnw

            def emit_consts():
                nc.sync.dma_start(idt[:], ident[:])
                nc.sync.dma_start(maskt[:], masks[:])
                # va default 1.0 -> per-head 65th column stays 1 (softmax
                # denominator rides the AV matmul); data columns are
                # overwritten by the tanh copies.
                for vg in range(4):
                    nc.sync.dma_start(
                        va_all[:, vg * 2080:(vg + 1) * 2080],
                        vinit[:, vg * 2080:(vg + 1) * 2080])
                for kg in range(4):
                    nc.sync.dma_start(wvf[:, 4 * kg:4 * kg + 4, :],
                                      wv4[:, 4 * kg:4 * kg + 4, :])
                if WITH_BIAS:
                    nc.sync.dma_start(bvt, bv[:])
                    nc.sync.dma_start(bqrow, bq[:])
                    nc.sync.dma_start(bkrow, bk[:])

            def qk_unit(w3, dall, brow, mt, half, xh, cosw, rsnw,
                        psa, paw, prt):
                wch = paw.tile([128, KT, 128], bf16, tag="wch", name="wch")
                nc.sync.dma_start(wch[:], w3[mt])
                ps = psa.tile([128, 2, 512], f32, tag="psa", name="ps_a")
                for qloc in range(2):
                    for k in range(KT):
                        nc.tensor.matmul(
                            ps[:, qloc, :], wch[:, k, :], xh[:, qloc, k, :],
                            start=(k == 0),
                            stop=(k == KT - 1 and not WITH_BIAS),
                        )
                    if WITH_BIAS:
                        nc.tensor.matmul(
                            ps[:, qloc, :],
                            brow[:, mt * 128:(mt + 1) * 128],
                            ones, start=False, stop=True,
                        )
                # RoPE: q*cos from PSUM into bf16 (cos table duplicated over
                # both head halves so the op is partition-aligned);
                # rotate-half*sin in f32 (partition-shifted ops must be
                # 4-byte), then an aligned mixed-dtype add.
                d3 = dall[:, mt, half * 1024:(half + 1) * 1024
                          ].rearrange("p (a b) -> p a b", a=2)
                tmp = prt.tile([128, 2, 512], f32, tag="tmp", name="tmp")
                nc.vector.tensor_mul(d3[:], ps[:], cosw[:])
                r2a, r2b = rsnw[0:32], rsnw[32:64]
                for b0 in (0, 64):
                    nc.vector.tensor_mul(
                        tmp[b0:b0 + 32], ps[b0 + 32:b0 + 64], r2a)
                    nc.vector.tensor_mul(
                        tmp[b0 + 32:b0 + 64], ps[b0:b0 + 32], r2b)
                nc.vector.tensor_add(d3[:], d3[:], tmp[:])

            def v_unit(qloc, st, half, xh, psv):
                qtr = 2 * half + qloc
                stg = qtr * 4 + st
                psb = psv.tile([128, CW], f32, tag="psv", name="ps_v")
                for k in range(KT):
                    nc.tensor.matmul(
                        psb[:], xh[:, qloc, k, st * 128:(st + 1) * 128],
                        wvf[:, k, :],
                        start=(k == 0),
                        stop=(k == KT - 1 and not WITH_BIAS),
                    )
                if WITH_BIAS:
                    nc.tensor.matmul(
                        psb[:], ones[:, 0:128], bvt, start=False, stop=True,
                    )
                # gate = tanh(v/2) = 2*sigmoid(v)-1; the 0.5 factor lives in
                # Wo and the +0.5 mean term is added on the host. One strided
                # activation covers all 8 heads.
                nc.scalar.activation(
                    va[stg].rearrange("p (h d) -> p h d", h=8)[:, :, 0:64],
                    psb[:].rearrange("p (h d) -> p h d", h=8),
                    Act.Tanh, scale=0.5,
                )

            def att_head(qb, pi, hh, pss, psy, pba, pbs):
                nkt = 4 * qb + 4
                h = 2 * pi + hh
                lo, hi = hh * 64, (hh + 1) * 64
                yps = psy.tile([65, 512], f32, tag="yps", name="ps_y")

                def _av(at2, kp):
                    for j in range(2):
                        k_i = 2 * kp + j
                        nc.tensor.matmul(
                            yps[:], va[k_i][:, 65 * h:65 * h + 65],
                            at2[:, j, :],
                            start=(k_i == 0), stop=(k_i == nkt - 1),
                        )

                prev = None
                for kp in range(nkt // 2):
                    ps2 = pss.tile([128, 2, 512], f32, tag="pss", name="ps_s")
                    for j in range(2):
                        k_i = 2 * kp + j
                        dt_i = k_i - 4 * qb
                        nc.tensor.matmul(
                            ps2[:, j, :],
                            kt[pi][lo:hi, k_i * 128:(k_i + 1) * 128],
                            qt[pi][lo:hi, qb * 512:(qb + 1) * 512],
                            start=True, stop=(dt_i < 0),
                        )
                        if dt_i >= 0:
                            w = 128 * (dt_i + 1)
                            nc.tensor.matmul(
                                ps2[:, j, 0:w], idt[:], maskt[:, dt_i, 0:w],
                                start=False, stop=True,
                            )
                    at2 = pba.tile([128, 2, 512], bf16, tag="at", name="at2")
                    nc.scalar.activation(at2[:], ps2[:], Act.Exp)
                    if prev is not None:
                        _av(*prev)
                        yield
                    prev = (at2, kp)
                _av(*prev)
                den = pbs.tile([1, 512], f32, tag="den", name="den")
                nc.vector.tensor_copy(den[:], yps[64:65, :])
                rc = pbs.tile([1, 512], f32, tag="rc", name="rc")
                nc.vector.reciprocal_approx_fast(rc[:], den[:])
                s128 = pbs.tile([128, 512], f32, tag="s128", name="s128")
                nc.gpsimd.partition_broadcast(s128[:], rc[:])
                nc.vector.tensor_mul(
                    ytr[pi][lo:hi, qb * 512:(qb + 1) * 512],
                    yps[0:64, :], s128[lo:hi, :],
                )
                yield

            def att_qbs(qbs, pss, psy, pba, pbs):
                for qb in qbs:
                    for pi in range(4):
                        for hh in range(2):
                            yield from att_head(qb, pi, hh, pss, psy,
                                                pba, pbs)

            def oproj(qb, pc, pso, pbo):
                for nt in range(4):
                    woc = pc.tile([128, 4, 512], bf16, tag="woc", name="woc")
                    nc.sync.dma_start(woc[:], wo4[nt])
                    for sl in range(4):
                        st = 4 * qb + sl
                        ps = pso.tile([128, 512], f32, tag="pso", name="ps_o")
                        for kc in range(4):
                            nc.tensor.matmul(
                                ps[:], ytr[kc][:, st * 128:(st + 1) * 128],
                                woc[:, kc, :],
                                start=(kc == 0), stop=(kc == 3),
                            )
                        ostg = pbo.tile([128, 512], f32, tag="ostg",
                                        name="ostg")
                        nc.vector.tensor_copy(ostg[:], ps[:])
                        nc.sync.dma_start(
                            part[st * 128:(st + 1) * 128,
                                 nt * 512:(nt + 1) * 512],
                            ostg[:],
                        )

            wsets = ((wq4, qt_all, bqrow), (wk4, kt_all, bkrow))

            with (
                tc.tile_pool(name="pa", bufs=1) as pa,
                tc.tile_pool(name="paw", bufs=3) as paw,
                tc.tile_pool(name="prc", bufs=1) as prc,
                tc.tile_pool(name="prt", bufs=2) as prt,
                tc.tile_pool(name="pba", bufs=3) as pba,
                tc.tile_pool(name="pbs", bufs=2) as pbs,
            ):
                # ---------------- half 0: q/k/v projections ----------------
                xh0 = load_xh(0, pa)
                cosw0, rsnw0 = load_rope(0, prc)
                emit_consts()
                with tc.tile_pool(name="psa", bufs=2, space="PSUM") as psa:
                    for w3, dall, brow in wsets:
                        for mt in range(4):
                            qk_unit(w3, dall, brow, mt, 0, xh0, cosw0, rsnw0,
                                    psa, paw, prt)
                with tc.tile_pool(name="psv", bufs=2, space="PSUM") as psv:
                    for qloc in range(2):
                        for st in range(4):
                            v_unit(qloc, st, 0, xh0, psv)

                # ---- half 1 interleaved with attention qb0/qb1 ----
                xh1 = load_xh(1, pa)
                cosw1, rsnw1 = load_rope(1, prc)
                with (
                    tc.tile_pool(name="pss1", bufs=1, space="PSUM") as pss1,
                    tc.tile_pool(name="psy1", bufs=2, space="PSUM") as psy1,
                ):
                    g = att_qbs((0, 1), pss1, psy1, pba, pbs)

                    def take(n):
                        for _ in range(n):
                            if next(g, _SENT) is _SENT:
                                return

                    with tc.tile_pool(name="psa2", bufs=2,
                                      space="PSUM") as psa2:
                        for w3, dall, brow in wsets:
                            for mt in range(4):
                                qk_unit(w3, dall, brow, mt, 1, xh1,
                                        cosw1, rsnw1, psa2, paw, prt)
                                take(3)
                    with tc.tile_pool(name="psv2", bufs=2,
                                      space="PSUM") as psv2:
                        for qloc in range(2):
                            for st in range(4):
                                v_unit(qloc, st, 1, xh1, psv2)
                                take(2)
                    for _ in g:
                        pass

            # ---------------- o-proj + attention qb2/qb3 ----------------
            with (
                tc.tile_pool(name="pc", bufs=2) as pc,
                tc.tile_pool(name="pbo", bufs=2) as pbo,
                tc.tile_pool(name="pba2", bufs=3) as pba2,
                tc.tile_pool(name="pbs2", bufs=2) as pbs2,
                tc.tile_pool(name="pssB", bufs=2, space="PSUM") as pssB,
                tc.tile_pool(name="psyB", bufs=2, space="PSUM") as psyB,
                tc.tile_pool(name="pso", bufs=2, space="PSUM") as pso,
            ):
                oproj(0, pc, pso, pbo)
                oproj(1, pc, pso, pbo)
                for qb in (2, 3):
                    for _ in att_qbs((qb,), pssB, psyB, pba2, pbs2):
                        pass
                    oproj(qb, pc, pso, pbo)

    nc.compile()
    return nc


def _rope_tables():
    half = DH // 2
    inv_freq = 1.0 / (10000.0 ** (np.arange(0, half, dtype=np.float32) / half))
    t = np.arange(S, dtype=np.float32)
    freqs = np.einsum("i,j->ij", t, inv_freq)            # [S, 32]
    emb = np.concatenate([freqs, freqs], axis=-1)        # [S, 64]
    cos = np.cos(emb).T.astype(np.float32)                        # [64, S]
    sin = np.sin(emb).T.astype(np.float32)
    rsin = np.concatenate([-sin[:32], sin[32:]], axis=0)
    return np.ascontiguousarray(np.concatenate([cos, rsin], axis=0))  # [128, S]


def _masks():
    j = np.arange(128)[:, None, None]
    dt = np.arange(4)[None, :, None]
    i = np.arange(512)[None, None, :]
    keep = (128 * dt + j) <= i
    return np.where(keep, 0.0, -1e30).astype(np.float32)  # [128, 4, 512]


def _bf(a):
    return np.ascontiguousarray(a).astype(ml_dtypes.bfloat16)


def kernel(**inputs):
    global LAST_EXEC_NS
    x = np.asarray(inputs["x"], dtype=np.float32)
    Wq = np.asarray(inputs["Wq"], dtype=np.float32)
    Wk = np.asarray(inputs["Wk"], dtype=np.float32)
    Wv = np.asarray(inputs["Wv"], dtype=np.float32)
    Wo = np.asarray(inputs["Wo"], dtype=np.float32)
    bq = np.asarray(inputs["bq"], dtype=np.float32)
    bk = np.asarray(inputs["bk"], dtype=np.float32)
    bv = np.asarray(inputs["bv"], dtype=np.float32)
    bo = np.asarray(inputs["bo"], dtype=np.float32)

    ropeT = _rope_tables()
    masks = _masks()

    with_bias = any(float(np.abs(b).max()) > 0 for b in (bq, bk, bv))
    nc = _build(WITH_BIAS=with_bias)
    cosT = np.concatenate([ropeT[0:64], ropeT[0:64]], axis=0)   # [128, S] dup
    ropec = np.ascontiguousarray(
        cosT.reshape(128, 4, 512).transpose(1, 0, 2))
    ropes = np.ascontiguousarray(
        ropeT[64:128].reshape(64, 4, 512).transpose(1, 0, 2))
    vinit = _bf(np.ones((128, 16 * 520), dtype=np.float32))
    ident = _bf(np.eye(128, dtype=np.float32))
    masks_b = _bf(masks)
    in_maps = []
    for c in range(N_CORES):
        b, g = c // 4, c % 4
        sl = slice(CW * g, CW * (g + 1))
        xT = x[b].T                                    # [D, S]
        x4 = _bf(xT.reshape(KT, 128, 4, 512).transpose(2, 1, 0, 3))
        wq4 = _bf(Wq[sl].T.reshape(KT, 128, 4, 128).transpose(2, 1, 0, 3))
        wk4 = _bf(
            (Wk[sl].T * 0.125).reshape(KT, 128, 4, 128).transpose(2, 1, 0, 3))
        wv4 = _bf(Wv[sl].T.reshape(KT, 128, CW).transpose(1, 0, 2))
        wo4 = _bf(
            (0.5 * Wo[:, sl]).T.reshape(4, 128, 4, 512).transpose(2, 1, 0, 3))
        in_maps.append({
            "x4": x4,
            "wq4": wq4,
            "wk4": wk4,
            "wv4": wv4,
            "wo4": wo4,
            "bq": _bf(bq[sl].reshape(1, CW)),
            "bk": _bf((bk[sl] * 0.125).reshape(1, CW)),
            "bv": _bf(bv[sl].reshape(1, CW)),
            "ropec": ropec,
            "ropes": ropes,
            "vinit": vinit,
            "ident": ident,
            "masks": masks_b,
        })

    kwargs = {}
    if TRACE:
        kwargs = dict(trace=True, trace_cores=list(range(N_CORES)),
                      stitch_traces=False)
        tdir = os.environ.get("KERNEL_TRACE_DIR")
        if tdir:
            os.makedirs(tdir, exist_ok=True)
            kwargs["tmpdir"] = tdir
    global LAST_MEAN_NS
    r = run_bass_kernel_spmd(nc, in_maps, list(range(N_CORES)), **kwargs)
    LAST_EXEC_NS = r.exec_time_ns
    LAST_MEAN_NS = r.mean_exec_time_ns

    # host "all-reduce": sum the 4 head-group partials per batch, add the
    # output bias and the 0.5*rowsum(Wo) term from the centered gate.
    const = bo + 0.5 * Wo.sum(axis=1)
    out = np.empty((B, S, D), dtype=np.float32)
    for b in range(B):
        acc = r.results[4 * b]["part"].astype(np.float32).copy()
        for g in range(1, 4):
            acc += r.results[4 * b + g]["part"]
        out[b] = acc + const
    return out
